# revision 1
# baseline (speedup 1.0000x reference)
"""HadamardLinear Trainium2 kernel.

Math (per token vector x in R^4096, G=32 groups of IO=128):
  y = beta * ( H^T @ ( (H^T @ (alpha * X)) @W_g grouped ) ) with H the
  normalized 32x32 Hadamard, W_g the per-group ternary-quantized weights.

Device pipeline (per core, 1024 tokens, bf16 compute / f32 accum+IO):
  T1 : PE stride-32-column transposes -> interleaved tiles X_j[g*4+i4, m]
  H1 : PE matmul, activations stationary, H(x)I4*alpha folded in the
       moving matrix -> token-major x_mixed (cols h*128+i)
  T2a: PE transposes of contiguous h-blocks -> feature-major Z_h[i, m]
  MM : PE matmul, activations stationary, exact ternary sign weights
       -> token-major y_parts (cols h*128+o)
  T2b: PE stride-32-column transposes -> Yint_k[h*4+o4, m]
  H2 : PE matmul, activations stationary, H*beta*scale/32 folded
       -> token-major f32 output
Sharding: data-parallel over the 8192 tokens across 8 cores; the three
128x4096 bf16 operand matrices are replicated.
"""

import sys

if "/opt/trn_rl_repo" not in sys.path:
    sys.path.insert(0, "/opt/trn_rl_repo")

import numpy as np
import ml_dtypes

BF16 = ml_dtypes.bfloat16

B, T, D = 4, 2048, 4096
G, IO = 32, 128
NCORES = 8
MC = (B * T) // NCORES      # tokens per core = 1024
CHUNK = 128                 # tokens per pipeline chunk
NCHUNK = MC // CHUNK        # 4
BLK = 128                   # token block (partition dim)
NBLK = CHUNK // BLK         # 2 blocks per chunk


def _hadamard_pm1(n):
    H = np.array([[1.0]], dtype=np.float32)
    while H.shape[0] < n:
        H = np.block([[H, H], [H, -H]])
    return H.astype(np.float32)


_NC_CACHE = {}


def _build_nc():
    if "nc" in _NC_CACHE:
        return _NC_CACHE["nc"]

    import concourse.bass as bass  # noqa: F401
    from concourse import bacc
    import concourse.tile as tile
    import concourse.mybir as mybir
    from concourse.masks import make_identity

    f32 = mybir.dt.float32
    bf16 = mybir.dt.bfloat16

    nc = bacc.Bacc("TRN2")
    x_p = nc.declare_dram_parameter("x", [MC, D], f32, isOutput=False)
    h1w_p = nc.declare_dram_parameter("h1w", [128, G * 128], bf16, isOutput=False)
    wt_p = nc.declare_dram_parameter("wt", [128, G * 128], bf16, isOutput=False)
    h2w_p = nc.declare_dram_parameter("h2w", [128, G * 128], bf16, isOutput=False)
    y_p = nc.declare_dram_parameter("y", [MC, D], f32, isOutput=True)

    with tile.TileContext(nc) as tc:
        with (
            tc.tile_pool(name="consts", bufs=1) as consts,
            tc.tile_pool(name="xin", bufs=3) as xin_pool,
            tc.tile_pool(name="xint", bufs=3) as xint_pool,
            tc.tile_pool(name="a4", bufs=3) as a4_pool,
            tc.tile_pool(name="z", bufs=3) as z_pool,
            tc.tile_pool(name="yp4", bufs=3) as yp4_pool,
            tc.tile_pool(name="yint", bufs=3) as yint_pool,
            tc.tile_pool(name="yout", bufs=2) as yout_pool,
            tc.tile_pool(name="ps_t1", bufs=3, space="PSUM") as ps_t1,
            tc.tile_pool(name="psf", bufs=4, space="PSUM") as psf,
        ):
            ident = consts.tile([128, 128], bf16)
            make_identity(nc, ident[:])

            h1w = consts.tile([128, G * 128], bf16)
            nc.sync.dma_start(out=h1w[:], in_=h1w_p[:])
            wt = consts.tile([128, G * 128], bf16)
            nc.sync.dma_start(out=wt[:], in_=wt_p[:])
            h2w = consts.tile([128, G * 128], bf16)
            nc.sync.dma_start(out=h2w[:], in_=h2w_p[:])

            def copy_engine(idx):
                return nc.vector.tensor_copy if idx % 2 == 0 else nc.scalar.copy

            for c in range(NCHUNK):
                # ---- T1: token-major -> interleaved feature-major ----
                # X free layout: (j, m) : j*CHUNK + m
                X = xint_pool.tile([128, G * CHUNK], bf16)
                for blk in range(NBLK):
                    xin = xin_pool.tile([128, D], bf16)
                    rows = c * CHUNK + blk * BLK
                    # SWDGE cast f32 -> bf16 during the load
                    nc.gpsimd.dma_start(out=xin[:], in_=x_p[rows : rows + BLK, :])
                    # cols {j + 32*t} give partition order p = g*4 + i4
                    # (single-stride AP as required for the stationary operand)
                    xv = xin.rearrange("m (g i4 ib) -> m ib (g i4)", g=G, i4=4)
                    for bank in range(4):
                        pt = ps_t1.tile([128, 8 * BLK], bf16, tag="pst")
                        for jj in range(8):
                            j = bank * 8 + jj
                            nc.tensor.transpose(
                                pt[:, jj * BLK : (jj + 1) * BLK],
                                xv[:, j, :],
                                ident[:],
                            )
                        dst = X.rearrange("p (j m) -> p j m", j=G)[
                            :, bank * 8 : (bank + 1) * 8, blk * BLK : (blk + 1) * BLK
                        ]
                        src = pt.rearrange("p (jj m) -> p jj m", jj=8)
                        copy_engine(bank + blk)(dst, src)

                # ---- H1 in lhsT form: token-major x_mixed, cols h*128+i ----
                xms = []
                for blk in range(NBLK):
                    xm = z_pool.tile([128, D], bf16)
                    xms.append(xm)
                    for jq in range(8):
                        ph = psf.tile([128, 512], f32, tag="psf")
                        for jj in range(4):
                            j = jq * 4 + jj
                            nc.tensor.matmul(
                                ph[:, jj * 128 : (jj + 1) * 128],
                                X[:, j * CHUNK + blk * BLK : j * CHUNK + (blk + 1) * BLK],
                                h1w[:, j * 128 : (j + 1) * 128],
                                start=True,
                                stop=True,
                            )
                        src = ph.rearrange("m (jj h i4) -> m jj h i4", jj=4, h=G)
                        dst = xm.rearrange("m (h i4 j) -> m j h i4", h=G, i4=4)[
                            :, jq * 4 : (jq + 1) * 4, :, :
                        ]
                        copy_engine(jq + blk)(dst, src)

                # ---- T2a: x_mixed -> feature-major Z_h[i, m] ----
                Z = yint_pool.tile([128, G * CHUNK], bf16)
                for blk in range(NBLK):
                    for hq in range(4):
                        pz = ps_t1.tile([128, 8 * BLK], bf16, tag="pst")
                        for hh in range(8):
                            h = hq * 8 + hh
                            nc.tensor.transpose(
                                pz[:, hh * BLK : (hh + 1) * BLK],
                                xms[blk][:, h * 128 : (h + 1) * 128],
                                ident[:],
                            )
                        dst = Z.rearrange("i (h m) -> i h m", h=G)[
                            :, hq * 8 : (hq + 1) * 8, blk * BLK : (blk + 1) * BLK
                        ]
                        src = pz.rearrange("i (hh m) -> i hh m", hh=8)
                        copy_engine(hq + blk + 1)(dst, src)

                # ---- MM in lhsT form: token-major y_parts, cols h*128+o ----
                yps = []
                for blk in range(NBLK):
                    yp = a4_pool.tile([128, D], bf16)
                    yps.append(yp)
                    for hq in range(8):
                        pm = psf.tile([128, 512], f32, tag="psf")
                        for hh in range(4):
                            h = hq * 4 + hh
                            nc.tensor.matmul(
                                pm[:, hh * 128 : (hh + 1) * 128],
                                Z[:, h * CHUNK + blk * BLK : h * CHUNK + (blk + 1) * BLK],
                                wt[:, h * 128 : (h + 1) * 128],
                                start=True,
                                stop=True,
                            )
                        # cols h*128+o are contiguous per h-block
                        copy_engine(hq + blk)(
                            yp[:, hq * 512 : (hq + 1) * 512], pm[:]
                        )

                # ---- T2b: y_parts -> Yint_k[p2=h*4+o4, m] (stride-32 cols) ----
                Yint = yp4_pool.tile([128, G * CHUNK], bf16)
                for blk in range(NBLK):
                    ypv = yps[blk].rearrange("m (h o4 kb) -> m kb (h o4)", h=G, o4=4)
                    for kq in range(4):
                        pyi = ps_t1.tile([128, 8 * BLK], bf16, tag="pst")
                        for kk in range(8):
                            k = kq * 8 + kk
                            nc.tensor.transpose(
                                pyi[:, kk * BLK : (kk + 1) * BLK],
                                ypv[:, k, :],
                                ident[:],
                            )
                        dst = Yint.rearrange("p (k m) -> p k m", k=G)[
                            :, kq * 8 : (kq + 1) * 8, blk * BLK : (blk + 1) * BLK
                        ]
                        src = pyi.rearrange("p (kk m) -> p kk m", kk=8)
                        copy_engine(kq + blk)(dst, src)

                # ---- H2 (token-major output) ----
                for blk in range(NBLK):
                    yout = yout_pool.tile([128, D], f32)
                    for kg in range(8):
                        p2 = psf.tile([128, 4 * 128], f32, tag="psf")
                        for kk in range(4):
                            k = kg * 4 + kk
                            nc.tensor.matmul(
                                p2[:, kk * 128 : (kk + 1) * 128],
                                Yint[:, k * CHUNK + blk * BLK : k * CHUNK + (blk + 1) * BLK],
                                h2w[:, k * 128 : (k + 1) * 128],
                                start=True,
                                stop=True,
                            )
                        src = p2.rearrange("p (kk hp o4p) -> p kk hp o4p", kk=4, hp=G)
                        dst = yout.rearrange(
                            "m (hp o4p k) -> m k hp o4p", hp=G, o4p=4
                        )[:, kg * 4 : (kg + 1) * 4, :, :]
                        copy_engine(kg + blk)(dst, src)
                    rows = c * CHUNK + blk * BLK
                    nc.sync.dma_start(out=y_p[rows : rows + BLK, :], in_=yout[:])

    nc.finalize()
    _NC_CACHE["nc"] = nc
    return nc


def _host_operands(weight, alpha, beta):
    """Fold quantization, Hadamards, alpha/beta into 3 device matrices."""
    Hr = _hadamard_pm1(G)  # +-1, exact in bf16
    scale = max(float(np.abs(weight).mean()), 1e-8)
    wq_sign = np.clip(np.round(weight / scale), -1.0, 1.0).astype(np.float32)
    S = scale / 32.0  # the two 1/sqrt(32) factors + ternary scale

    # h1w[p=(g*4+i4), j, q=(h*4+i4p)] = delta(i4,i4p) * Hr[g,h] * alpha[g, i4*32+j]
    h1w = np.zeros((G, 4, G, G, 4), dtype=np.float32)  # g, i4, j, h, i4p
    for i4 in range(4):
        a = alpha[:, i4 * 32 : (i4 + 1) * 32].astype(np.float32)  # [g, j]
        h1w[:, i4, :, :, i4] = a[:, :, None] * Hr[:, None, :]
    h1w = h1w.reshape(128, G, 128).reshape(128, G * 128).astype(BF16)

    # wt[i, h, o] = wq_sign[h, o, i]
    wt = np.ascontiguousarray(np.transpose(wq_sign, (2, 0, 1))).reshape(
        128, G * 128
    ).astype(BF16)

    # h2w[p'=(o4*32+h), k, q'=(hp*4+o4p)] = delta(o4,o4p)*Hr[h,hp]*beta[hp,o4*32+k]*S
    h2w = np.zeros((4, G, G, G, 4), dtype=np.float32)  # o4, h, k, hp, o4p
    for o4 in range(4):
        b = beta[:, o4 * 32 : (o4 + 1) * 32].astype(np.float32) * S  # [hp, k]
        # Hr[h, hp] * b[hp, k] -> [h, k, hp]
        h2w[o4, :, :, :, o4] = Hr[:, None, :] * b.T[None, :, :]
    # device rows use p2 = h*4 + o4 (T2b transpose partition order)
    h2w = np.transpose(h2w, (1, 0, 2, 3, 4)).reshape(128, G, 128)
    h2w = h2w.reshape(128, G * 128).astype(BF16)
    return h1w, wt, h2w


def _run(x, weight, alpha, beta, trace=False, **spmd_kwargs):
    from concourse.bass_utils import run_bass_kernel_spmd

    x = np.asarray(x, dtype=np.float32)
    h1w, wt, h2w = _host_operands(
        np.asarray(weight, dtype=np.float32),
        np.asarray(alpha, dtype=np.float32),
        np.asarray(beta, dtype=np.float32),
    )

    x2 = np.ascontiguousarray(x.reshape(B * T, D))
    in_maps = [
        {
            "x": np.ascontiguousarray(x2[c * MC : (c + 1) * MC]),
            "h1w": h1w,
            "wt": wt,
            "h2w": h2w,
        }
        for c in range(NCORES)
    ]
    nc = _build_nc()
    res = run_bass_kernel_spmd(
        nc, in_maps, list(range(NCORES)), trace=trace, **spmd_kwargs
    )
    y = np.concatenate([res.results[c]["y"] for c in range(NCORES)], axis=0)
    return np.ascontiguousarray(y.reshape(B, T, D).astype(np.float32)), res


def kernel(x, weight, alpha, beta):
    y, _ = _run(x, weight, alpha, beta)
    return y



# revision 2
# speedup vs baseline: 2.3519x; 2.3519x over previous
"""HadamardLinear Trainium2 kernel.

Math (per token vector x in R^4096, G=32 groups of IO=128):
  y = beta * ( H^T @ ( (H^T @ (alpha * X)) @W_g grouped ) ) with H the
  normalized 32x32 Hadamard, W_g the per-group ternary-quantized weights.

Device pipeline (per core, 1024 tokens, fp16 compute / f32 accum):
  T1 : PE stride-32-column transposes -> interleaved tiles X_j[g*4+i4, m]
  H1 : PE matmul, activations stationary, H(x)I4*alpha folded in the
       moving matrix -> token-major x_mixed (cols h*128+i)
  T2a: PE transposes of contiguous h-blocks -> feature-major Z_h[i, m]
  MM : PE matmul, activations stationary, exact ternary sign weights
       -> token-major y_parts (cols h*128+o)
  T2b: PE stride-32-column transposes -> Yint_k[h*4+o4, m]
  H2 : PE matmul, activations stationary, H*beta*scale/32 folded
       -> token-major fp16 output

Sharding: data-parallel over the 8192 tokens across 8 cores; the three
128x4096 fp16 operand matrices are replicated.

Host runner: the wall-clock cost of a call is dominated by the axon
tunnel (~50-70 MB/s H2D, ~45 MB/s D2H, serialized), so the runner
 - ships x and y as fp16 (halves the wire bytes vs f32),
 - passes full global arrays into a shard_map'd executable built ONCE
   (no per-call re-trace / re-lower / XLA compile),
 - reuses a cached device-resident dummy buffer for the output operand
   instead of uploading 134 MB of zeros per call (the kernel writes
   every element of y).
"""

import sys

if "/opt/trn_rl_repo" not in sys.path:
    sys.path.insert(0, "/opt/trn_rl_repo")

import numpy as np

B, T, D = 4, 2048, 4096
G, IO = 32, 128
NCORES = 8
MC = (B * T) // NCORES      # tokens per core = 1024
CHUNK = 128                 # tokens per pipeline chunk
NCHUNK = MC // CHUNK        # 8
BLK = 128                   # token block (partition dim)
NBLK = CHUNK // BLK         # 1 block per chunk


def _hadamard_pm1(n):
    H = np.array([[1.0]], dtype=np.float32)
    while H.shape[0] < n:
        H = np.block([[H, H], [H, -H]])
    return H.astype(np.float32)


_CACHE = {}


def _build_nc():
    if "nc" in _CACHE:
        return _CACHE["nc"]

    import concourse.bass as bass  # noqa: F401
    from concourse import bacc
    import concourse.tile as tile
    import concourse.mybir as mybir
    from concourse.masks import make_identity

    f32 = mybir.dt.float32
    f16 = mybir.dt.float16

    nc = bacc.Bacc("TRN2")
    x_p = nc.declare_dram_parameter("x", [MC, D], f16, isOutput=False)
    h1w_p = nc.declare_dram_parameter("h1w", [128, G * 128], f16, isOutput=False)
    wt_p = nc.declare_dram_parameter("wt", [128, G * 128], f16, isOutput=False)
    h2w_p = nc.declare_dram_parameter("h2w", [128, G * 128], f16, isOutput=False)
    y_p = nc.declare_dram_parameter("y", [MC, D], f16, isOutput=True)

    with tile.TileContext(nc) as tc:
        with (
            tc.tile_pool(name="consts", bufs=1) as consts,
            tc.tile_pool(name="xin", bufs=3) as xin_pool,
            tc.tile_pool(name="xint", bufs=3) as xint_pool,
            tc.tile_pool(name="a4", bufs=3) as a4_pool,
            tc.tile_pool(name="z", bufs=3) as z_pool,
            tc.tile_pool(name="yp4", bufs=3) as yp4_pool,
            tc.tile_pool(name="yint", bufs=3) as yint_pool,
            tc.tile_pool(name="yout", bufs=2) as yout_pool,
            tc.tile_pool(name="ps_t1", bufs=3, space="PSUM") as ps_t1,
            tc.tile_pool(name="psf", bufs=4, space="PSUM") as psf,
        ):
            ident = consts.tile([128, 128], f16)
            make_identity(nc, ident[:])

            h1w = consts.tile([128, G * 128], f16)
            nc.sync.dma_start(out=h1w[:], in_=h1w_p[:])
            wt = consts.tile([128, G * 128], f16)
            nc.sync.dma_start(out=wt[:], in_=wt_p[:])
            h2w = consts.tile([128, G * 128], f16)
            nc.sync.dma_start(out=h2w[:], in_=h2w_p[:])

            def copy_engine(idx):
                return nc.vector.tensor_copy if idx % 2 == 0 else nc.scalar.copy

            for c in range(NCHUNK):
                # ---- T1: token-major -> interleaved feature-major ----
                # X free layout: (j, m) : j*CHUNK + m
                X = xint_pool.tile([128, G * CHUNK], f16)
                for blk in range(NBLK):
                    xin = xin_pool.tile([128, D], f16)
                    rows = c * CHUNK + blk * BLK
                    nc.sync.dma_start(out=xin[:], in_=x_p[rows : rows + BLK, :])
                    # cols {j + 32*t} give partition order p = g*4 + i4
                    # (single-stride AP as required for the stationary operand)
                    xv = xin.rearrange("m (g i4 ib) -> m ib (g i4)", g=G, i4=4)
                    for bank in range(4):
                        pt = ps_t1.tile([128, 8 * BLK], f16, tag="pst")
                        for jj in range(8):
                            j = bank * 8 + jj
                            nc.tensor.transpose(
                                pt[:, jj * BLK : (jj + 1) * BLK],
                                xv[:, j, :],
                                ident[:],
                            )
                        dst = X.rearrange("p (j m) -> p j m", j=G)[
                            :, bank * 8 : (bank + 1) * 8, blk * BLK : (blk + 1) * BLK
                        ]
                        src = pt.rearrange("p (jj m) -> p jj m", jj=8)
                        copy_engine(bank + blk)(dst, src)

                # ---- H1 in lhsT form: token-major x_mixed, cols h*128+i ----
                xms = []
                for blk in range(NBLK):
                    xm = z_pool.tile([128, D], f16)
                    xms.append(xm)
                    for jq in range(8):
                        ph = psf.tile([128, 512], f32, tag="psf")
                        for jj in range(4):
                            j = jq * 4 + jj
                            nc.tensor.matmul(
                                ph[:, jj * 128 : (jj + 1) * 128],
                                X[:, j * CHUNK + blk * BLK : j * CHUNK + (blk + 1) * BLK],
                                h1w[:, j * 128 : (j + 1) * 128],
                                start=True,
                                stop=True,
                            )
                        src = ph.rearrange("m (jj h i4) -> m jj h i4", jj=4, h=G)
                        dst = xm.rearrange("m (h i4 j) -> m j h i4", h=G, i4=4)[
                            :, jq * 4 : (jq + 1) * 4, :, :
                        ]
                        copy_engine(jq + blk)(dst, src)

                # ---- T2a: x_mixed -> feature-major Z_h[i, m] ----
                Z = yint_pool.tile([128, G * CHUNK], f16)
                for blk in range(NBLK):
                    for hq in range(4):
                        pz = ps_t1.tile([128, 8 * BLK], f16, tag="pst")
                        for hh in range(8):
                            h = hq * 8 + hh
                            nc.tensor.transpose(
                                pz[:, hh * BLK : (hh + 1) * BLK],
                                xms[blk][:, h * 128 : (h + 1) * 128],
                                ident[:],
                            )
                        dst = Z.rearrange("i (h m) -> i h m", h=G)[
                            :, hq * 8 : (hq + 1) * 8, blk * BLK : (blk + 1) * BLK
                        ]
                        src = pz.rearrange("i (hh m) -> i hh m", hh=8)
                        copy_engine(hq + blk + 1)(dst, src)

                # ---- MM in lhsT form: token-major y_parts, cols h*128+o ----
                yps = []
                for blk in range(NBLK):
                    yp = a4_pool.tile([128, D], f16)
                    yps.append(yp)
                    for hq in range(8):
                        pm = psf.tile([128, 512], f32, tag="psf")
                        for hh in range(4):
                            h = hq * 4 + hh
                            nc.tensor.matmul(
                                pm[:, hh * 128 : (hh + 1) * 128],
                                Z[:, h * CHUNK + blk * BLK : h * CHUNK + (blk + 1) * BLK],
                                wt[:, h * 128 : (h + 1) * 128],
                                start=True,
                                stop=True,
                            )
                        # cols h*128+o are contiguous per h-block
                        copy_engine(hq + blk)(
                            yp[:, hq * 512 : (hq + 1) * 512], pm[:]
                        )

                # ---- T2b: y_parts -> Yint_k[p2=h*4+o4, m] (stride-32 cols) ----
                Yint = yp4_pool.tile([128, G * CHUNK], f16)
                for blk in range(NBLK):
                    ypv = yps[blk].rearrange("m (h o4 kb) -> m kb (h o4)", h=G, o4=4)
                    for kq in range(4):
                        pyi = ps_t1.tile([128, 8 * BLK], f16, tag="pst")
                        for kk in range(8):
                            k = kq * 8 + kk
                            nc.tensor.transpose(
                                pyi[:, kk * BLK : (kk + 1) * BLK],
                                ypv[:, k, :],
                                ident[:],
                            )
                        dst = Yint.rearrange("p (k m) -> p k m", k=G)[
                            :, kq * 8 : (kq + 1) * 8, blk * BLK : (blk + 1) * BLK
                        ]
                        src = pyi.rearrange("p (kk m) -> p kk m", kk=8)
                        copy_engine(kq + blk)(dst, src)

                # ---- H2 (token-major output) ----
                for blk in range(NBLK):
                    yout = yout_pool.tile([128, D], f16)
                    for kg in range(8):
                        p2 = psf.tile([128, 4 * 128], f32, tag="psf")
                        for kk in range(4):
                            k = kg * 4 + kk
                            nc.tensor.matmul(
                                p2[:, kk * 128 : (kk + 1) * 128],
                                Yint[:, k * CHUNK + blk * BLK : k * CHUNK + (blk + 1) * BLK],
                                h2w[:, k * 128 : (k + 1) * 128],
                                start=True,
                                stop=True,
                            )
                        src = p2.rearrange("p (kk hp o4p) -> p kk hp o4p", kk=4, hp=G)
                        dst = yout.rearrange(
                            "m (hp o4p k) -> m k hp o4p", hp=G, o4p=4
                        )[:, kg * 4 : (kg + 1) * 4, :, :]
                        copy_engine(kg + blk)(dst, src)
                    rows = c * CHUNK + blk * BLK
                    nc.sync.dma_start(out=y_p[rows : rows + BLK, :], in_=yout[:])

    nc.finalize()
    _CACHE["nc"] = nc
    return nc


def _host_operands(weight, alpha, beta):
    """Fold quantization, Hadamards, alpha/beta into 3 device matrices."""
    Hr = _hadamard_pm1(G)  # +-1, exact in fp16
    scale = max(float(np.abs(weight).mean()), 1e-8)
    wq_sign = np.clip(np.round(weight / scale), -1.0, 1.0).astype(np.float32)
    S = scale / 32.0  # the two 1/sqrt(32) factors + ternary scale

    # h1w[p=(g*4+i4), j, q=(h*4+i4p)] = delta(i4,i4p) * Hr[g,h] * alpha[g, i4*32+j]
    h1w = np.zeros((G, 4, G, G, 4), dtype=np.float32)  # g, i4, j, h, i4p
    for i4 in range(4):
        a = alpha[:, i4 * 32 : (i4 + 1) * 32].astype(np.float32)  # [g, j]
        h1w[:, i4, :, :, i4] = a[:, :, None] * Hr[:, None, :]
    h1w = h1w.reshape(128, G * 128).astype(np.float16)

    # wt[i, h, o] = wq_sign[h, o, i]
    wt = np.ascontiguousarray(np.transpose(wq_sign, (2, 0, 1))).reshape(
        128, G * 128
    ).astype(np.float16)

    # h2w[p'=(o4*32+h), k, q'=(hp*4+o4p)] = delta(o4,o4p)*Hr[h,hp]*beta[hp,o4*32+k]*S
    h2w = np.zeros((4, G, G, G, 4), dtype=np.float32)  # o4, h, k, hp, o4p
    for o4 in range(4):
        b = beta[:, o4 * 32 : (o4 + 1) * 32].astype(np.float32) * S  # [hp, k]
        # Hr[h, hp] * b[hp, k] -> [h, k, hp]
        h2w[o4, :, :, :, o4] = Hr[:, None, :] * b.T[None, :, :]
    # device rows use p2 = h*4 + o4 (T2b transpose partition order)
    h2w = np.transpose(h2w, (1, 0, 2, 3, 4)).reshape(128, G * 128).astype(np.float16)
    return h1w, wt, h2w


class _Result:
    """Minimal result shim for test harnesses (no NTFF trace available)."""

    exec_time_ns = None
    mean_exec_time_ns = None


def _get_runtime():
    """Build (once) the jitted shard_map executable and cached device buffers."""
    if "rt" in _CACHE:
        return _CACHE["rt"]

    nc = _build_nc()

    import jax
    from jax.sharding import Mesh, PartitionSpec, NamedSharding
    from jax.experimental.shard_map import shard_map
    from concourse.bass2jax import (
        _bass_exec_p,
        install_neuronx_cc_hook,
        partition_id_tensor,
    )
    import concourse.mybir as mybir

    install_neuronx_cc_hook()

    partition_name = (
        nc.partition_id_tensor.name if nc.partition_id_tensor else None
    )
    in_names, out_names, out_avals = [], [], []
    for alloc in nc.m.functions[0].allocations:
        if not isinstance(alloc, mybir.MemoryLocationSet):
            continue
        name = alloc.memorylocations[0].name
        if alloc.kind == "ExternalInput":
            if name != partition_name:
                in_names.append(name)
        elif alloc.kind == "ExternalOutput":
            out_names.append(name)
            out_avals.append(
                jax.core.ShapedArray(
                    tuple(alloc.tensor_shape), mybir.dt.np(alloc.dtype)
                )
            )
    n_params = len(in_names)
    all_in_names = tuple(in_names + out_names + ([partition_name] if partition_name else []))

    def _body(*args):
        operands = list(args)
        if partition_name is not None:
            operands.append(partition_id_tensor())
        outs = _bass_exec_p.bind(
            *operands,
            out_avals=tuple(out_avals),
            in_names=all_in_names,
            out_names=tuple(out_names),
            lowering_input_output_aliases=(),
            sim_require_finite=True,
            sim_require_nnan=True,
            nc=nc,
        )
        return tuple(outs)

    devices = jax.devices()[:NCORES]
    mesh = Mesh(np.asarray(devices), ("core",))
    shard = NamedSharding(mesh, PartitionSpec("core"))
    n_args = n_params + len(out_names)
    fn = jax.jit(
        shard_map(
            _body,
            mesh=mesh,
            in_specs=(PartitionSpec("core"),) * n_args,
            out_specs=(PartitionSpec("core"),) * len(out_names),
            check_rep=False,
        ),
        keep_unused=True,
    )

    # Cached device-resident operand for the output slot: its content is
    # never observed (the kernel writes every element of y), so one
    # buffer is uploaded once and reused by every call.
    ybuf = jax.device_put(np.zeros((NCORES * MC, D), np.float16), shard)
    jax.block_until_ready(ybuf)

    rt = {
        "fn": fn,
        "shard": shard,
        "ybuf": ybuf,
        "in_names": in_names,
        "jax": jax,
    }
    _CACHE["rt"] = rt
    return rt


def _run(x, weight, alpha, beta, trace=False, **spmd_kwargs):
    rt = _get_runtime()

    h1w, wt, h2w = _host_operands(
        np.asarray(weight, dtype=np.float32),
        np.asarray(alpha, dtype=np.float32),
        np.asarray(beta, dtype=np.float32),
    )
    x16 = np.asarray(x, dtype=np.float32).reshape(B * T, D).astype(np.float16)
    ops = {
        "x": x16,
        "h1w": np.tile(h1w, (NCORES, 1)),
        "wt": np.tile(wt, (NCORES, 1)),
        "h2w": np.tile(h2w, (NCORES, 1)),
    }
    args = [ops[nm] for nm in rt["in_names"]]
    out = rt["fn"](*args, rt["ybuf"])
    y16 = np.asarray(out[0])
    y = y16.astype(np.float32).reshape(B, T, D)
    return y, _Result()


def kernel(x, weight, alpha, beta):
    y, _ = _run(x, weight, alpha, beta)
    return y


# revision 3
# speedup vs baseline: 3.2020x; 1.3615x over previous
"""HadamardLinear Trainium2 kernel.

Math (per token vector x in R^4096, G=32 groups of IO=128):
  y = beta * ( H^T @ ( (H^T @ (alpha * X)) @W_g grouped ) ) with H the
  normalized 32x32 Hadamard, W_g the per-group ternary-quantized weights.

Device pipeline (per core, RC tokens per exec, fp16 compute / f32 accum):
  T1 : PE stride-32-column transposes -> interleaved tiles X_j[g*4+i4, m]
  H1 : PE matmul, activations stationary, H(x)I4*alpha folded in the
       moving matrix -> token-major x_mixed (cols h*128+i)
  T2a: PE transposes of contiguous h-blocks -> feature-major Z_h[i, m]
  MM : PE matmul, activations stationary, exact ternary sign weights
       -> token-major y_parts (cols h*128+o)
  T2b: PE stride-32-column transposes -> Yint_k[h*4+o4, m]
  H2 : PE matmul, activations stationary, H*beta*scale/32 folded
       -> token-major fp16 output

Sharding: data-parallel over the 8192 tokens across 8 cores; the three
128x4096 fp16 operand matrices (stacked into one 384x4096 tensor) are
replicated.

Host runner notes: a call's wall time is dominated by the axon tunnel
(~50 MB/s H2D, ~43 MB/s D2H, partially full-duplex), so the runner
 - ships x and y as fp16 (halves the wire bytes vs f32),
 - uses a jitted shard_map executable built ONCE (no per-call re-trace
   / re-lower / XLA compile),
 - uploads the 3 MB folded weight stack to core 0 only and broadcasts
   it device-to-device (terminal-side, ~free) instead of 8x over the
   tunnel,
 - reuses a cached device-resident dummy buffer for the output operand
   (the kernel writes every element of y, so its content is never
   observed),
 - splits the 8192 tokens into NCH sequential execs so the download of
   chunk i overlaps the upload of chunk i+1.
"""

import sys

if "/opt/trn_rl_repo" not in sys.path:
    sys.path.insert(0, "/opt/trn_rl_repo")

import numpy as np

B, T, D = 4, 2048, 4096
G, IO = 32, 128
NCORES = 8
MC = (B * T) // NCORES      # tokens per core = 1024
RC = 256                    # tokens per core per exec-chunk
NCH = MC // RC              # 4 transfer/exec chunks per call
CHUNK = 128                 # tokens per pipeline chunk
BLK = 128                   # token block (partition dim)
NBLK = 1


def _hadamard_pm1(n):
    H = np.array([[1.0]], dtype=np.float32)
    while H.shape[0] < n:
        H = np.block([[H, H], [H, -H]])
    return H.astype(np.float32)


_CACHE = {}


def _build_nc():
    if "nc" in _CACHE:
        return _CACHE["nc"]

    import concourse.bass as bass  # noqa: F401
    from concourse import bacc
    import concourse.tile as tile
    import concourse.mybir as mybir
    from concourse.masks import make_identity

    f32 = mybir.dt.float32
    f16 = mybir.dt.float16

    nc = bacc.Bacc("TRN2")
    x_p = nc.declare_dram_parameter("x", [RC, D], f16, isOutput=False)
    w3_p = nc.declare_dram_parameter("w3", [3 * 128, G * 128], f16, isOutput=False)
    y_p = nc.declare_dram_parameter("y", [RC, D], f16, isOutput=True)

    with tile.TileContext(nc) as tc:
        with (
            tc.tile_pool(name="consts", bufs=1) as consts,
            tc.tile_pool(name="xin", bufs=3) as xin_pool,
            tc.tile_pool(name="xint", bufs=3) as xint_pool,
            tc.tile_pool(name="a4", bufs=3) as a4_pool,
            tc.tile_pool(name="z", bufs=3) as z_pool,
            tc.tile_pool(name="yp4", bufs=3) as yp4_pool,
            tc.tile_pool(name="yint", bufs=3) as yint_pool,
            tc.tile_pool(name="yout", bufs=2) as yout_pool,
            tc.tile_pool(name="ps_t1", bufs=3, space="PSUM") as ps_t1,
            tc.tile_pool(name="psf", bufs=4, space="PSUM") as psf,
        ):
            ident = consts.tile([128, 128], f16)
            make_identity(nc, ident[:])

            h1w = consts.tile([128, G * 128], f16)
            nc.sync.dma_start(out=h1w[:], in_=w3_p[0:128, :])
            wt = consts.tile([128, G * 128], f16)
            nc.sync.dma_start(out=wt[:], in_=w3_p[128:256, :])
            h2w = consts.tile([128, G * 128], f16)
            nc.sync.dma_start(out=h2w[:], in_=w3_p[256:384, :])

            def copy_engine(idx):
                return nc.vector.tensor_copy if idx % 2 == 0 else nc.scalar.copy

            for c in range(RC // CHUNK):
                # ---- T1: token-major -> interleaved feature-major ----
                # X free layout: (j, m) : j*CHUNK + m
                X = xint_pool.tile([128, G * CHUNK], f16)
                for blk in range(NBLK):
                    xin = xin_pool.tile([128, D], f16)
                    rows = c * CHUNK + blk * BLK
                    nc.sync.dma_start(out=xin[:], in_=x_p[rows : rows + BLK, :])
                    # cols {j + 32*t} give partition order p = g*4 + i4
                    # (single-stride AP as required for the stationary operand)
                    xv = xin.rearrange("m (g i4 ib) -> m ib (g i4)", g=G, i4=4)
                    for bank in range(4):
                        pt = ps_t1.tile([128, 8 * BLK], f16, tag="pst")
                        for jj in range(8):
                            j = bank * 8 + jj
                            nc.tensor.transpose(
                                pt[:, jj * BLK : (jj + 1) * BLK],
                                xv[:, j, :],
                                ident[:],
                            )
                        dst = X.rearrange("p (j m) -> p j m", j=G)[
                            :, bank * 8 : (bank + 1) * 8, blk * BLK : (blk + 1) * BLK
                        ]
                        src = pt.rearrange("p (jj m) -> p jj m", jj=8)
                        copy_engine(bank + blk)(dst, src)

                # ---- H1 in lhsT form: token-major x_mixed, cols h*128+i ----
                xms = []
                for blk in range(NBLK):
                    xm = z_pool.tile([128, D], f16)
                    xms.append(xm)
                    for jq in range(8):
                        ph = psf.tile([128, 512], f32, tag="psf")
                        for jj in range(4):
                            j = jq * 4 + jj
                            nc.tensor.matmul(
                                ph[:, jj * 128 : (jj + 1) * 128],
                                X[:, j * CHUNK + blk * BLK : j * CHUNK + (blk + 1) * BLK],
                                h1w[:, j * 128 : (j + 1) * 128],
                                start=True,
                                stop=True,
                            )
                        src = ph.rearrange("m (jj h i4) -> m jj h i4", jj=4, h=G)
                        dst = xm.rearrange("m (h i4 j) -> m j h i4", h=G, i4=4)[
                            :, jq * 4 : (jq + 1) * 4, :, :
                        ]
                        copy_engine(jq + blk)(dst, src)

                # ---- T2a: x_mixed -> feature-major Z_h[i, m] ----
                Z = yint_pool.tile([128, G * CHUNK], f16)
                for blk in range(NBLK):
                    for hq in range(4):
                        pz = ps_t1.tile([128, 8 * BLK], f16, tag="pst")
                        for hh in range(8):
                            h = hq * 8 + hh
                            nc.tensor.transpose(
                                pz[:, hh * BLK : (hh + 1) * BLK],
                                xms[blk][:, h * 128 : (h + 1) * 128],
                                ident[:],
                            )
                        dst = Z.rearrange("i (h m) -> i h m", h=G)[
                            :, hq * 8 : (hq + 1) * 8, blk * BLK : (blk + 1) * BLK
                        ]
                        src = pz.rearrange("i (hh m) -> i hh m", hh=8)
                        copy_engine(hq + blk + 1)(dst, src)

                # ---- MM in lhsT form: token-major y_parts, cols h*128+o ----
                yps = []
                for blk in range(NBLK):
                    yp = a4_pool.tile([128, D], f16)
                    yps.append(yp)
                    for hq in range(8):
                        pm = psf.tile([128, 512], f32, tag="psf")
                        for hh in range(4):
                            h = hq * 4 + hh
                            nc.tensor.matmul(
                                pm[:, hh * 128 : (hh + 1) * 128],
                                Z[:, h * CHUNK + blk * BLK : h * CHUNK + (blk + 1) * BLK],
                                wt[:, h * 128 : (h + 1) * 128],
                                start=True,
                                stop=True,
                            )
                        # cols h*128+o are contiguous per h-block
                        copy_engine(hq + blk)(
                            yp[:, hq * 512 : (hq + 1) * 512], pm[:]
                        )

                # ---- T2b: y_parts -> Yint_k[p2=h*4+o4, m] (stride-32 cols) ----
                Yint = yp4_pool.tile([128, G * CHUNK], f16)
                for blk in range(NBLK):
                    ypv = yps[blk].rearrange("m (h o4 kb) -> m kb (h o4)", h=G, o4=4)
                    for kq in range(4):
                        pyi = ps_t1.tile([128, 8 * BLK], f16, tag="pst")
                        for kk in range(8):
                            k = kq * 8 + kk
                            nc.tensor.transpose(
                                pyi[:, kk * BLK : (kk + 1) * BLK],
                                ypv[:, k, :],
                                ident[:],
                            )
                        dst = Yint.rearrange("p (k m) -> p k m", k=G)[
                            :, kq * 8 : (kq + 1) * 8, blk * BLK : (blk + 1) * BLK
                        ]
                        src = pyi.rearrange("p (kk m) -> p kk m", kk=8)
                        copy_engine(kq + blk)(dst, src)

                # ---- H2 (token-major output) ----
                for blk in range(NBLK):
                    yout = yout_pool.tile([128, D], f16)
                    for kg in range(8):
                        p2 = psf.tile([128, 4 * 128], f32, tag="psf")
                        for kk in range(4):
                            k = kg * 4 + kk
                            nc.tensor.matmul(
                                p2[:, kk * 128 : (kk + 1) * 128],
                                Yint[:, k * CHUNK + blk * BLK : k * CHUNK + (blk + 1) * BLK],
                                h2w[:, k * 128 : (k + 1) * 128],
                                start=True,
                                stop=True,
                            )
                        src = p2.rearrange("p (kk hp o4p) -> p kk hp o4p", kk=4, hp=G)
                        dst = yout.rearrange(
                            "m (hp o4p k) -> m k hp o4p", hp=G, o4p=4
                        )[:, kg * 4 : (kg + 1) * 4, :, :]
                        copy_engine(kg + blk)(dst, src)
                    rows = c * CHUNK + blk * BLK
                    nc.sync.dma_start(out=y_p[rows : rows + BLK, :], in_=yout[:])

    nc.finalize()
    _CACHE["nc"] = nc
    return nc


def _host_operands(weight, alpha, beta):
    """Fold quantization, Hadamards, alpha/beta into 3 device matrices."""
    Hr = _hadamard_pm1(G)  # +-1, exact in fp16
    scale = max(float(np.abs(weight).mean()), 1e-8)
    wq_sign = np.clip(np.round(weight / scale), -1.0, 1.0).astype(np.float32)
    S = scale / 32.0  # the two 1/sqrt(32) factors + ternary scale

    # h1w[p=(g*4+i4), j, q=(h*4+i4p)] = delta(i4,i4p) * Hr[g,h] * alpha[g, i4*32+j]
    h1w = np.zeros((G, 4, G, G, 4), dtype=np.float32)  # g, i4, j, h, i4p
    for i4 in range(4):
        a = alpha[:, i4 * 32 : (i4 + 1) * 32].astype(np.float32)  # [g, j]
        h1w[:, i4, :, :, i4] = a[:, :, None] * Hr[:, None, :]
    h1w = h1w.reshape(128, G * 128)

    # wt[i, h, o] = wq_sign[h, o, i]
    wt = np.ascontiguousarray(np.transpose(wq_sign, (2, 0, 1))).reshape(128, G * 128)

    # h2w[p'=(o4*32+h), k, q'=(hp*4+o4p)] = delta(o4,o4p)*Hr[h,hp]*beta[hp,o4*32+k]*S
    h2w = np.zeros((4, G, G, G, 4), dtype=np.float32)  # o4, h, k, hp, o4p
    for o4 in range(4):
        b = beta[:, o4 * 32 : (o4 + 1) * 32].astype(np.float32) * S  # [hp, k]
        # Hr[h, hp] * b[hp, k] -> [h, k, hp]
        h2w[o4, :, :, :, o4] = Hr[:, None, :] * b.T[None, :, :]
    # device rows use p2 = h*4 + o4 (T2b transpose partition order)
    h2w = np.transpose(h2w, (1, 0, 2, 3, 4)).reshape(128, G * 128)

    w3 = np.concatenate([h1w, wt, h2w], axis=0).astype(np.float16)
    return w3


class _Result:
    """Minimal result shim for test harnesses (no NTFF trace available)."""

    exec_time_ns = None
    mean_exec_time_ns = None


def _get_runtime():
    """Build (once) the jitted shard_map executable and cached device buffers."""
    if "rt" in _CACHE:
        return _CACHE["rt"]

    nc = _build_nc()

    import jax
    from jax.sharding import Mesh, PartitionSpec, NamedSharding
    from jax.experimental.shard_map import shard_map
    from concourse.bass2jax import (
        _bass_exec_p,
        install_neuronx_cc_hook,
        partition_id_tensor,
    )
    import concourse.mybir as mybir

    install_neuronx_cc_hook()

    partition_name = (
        nc.partition_id_tensor.name if nc.partition_id_tensor else None
    )
    in_names, out_names, out_avals = [], [], []
    for alloc in nc.m.functions[0].allocations:
        if not isinstance(alloc, mybir.MemoryLocationSet):
            continue
        name = alloc.memorylocations[0].name
        if alloc.kind == "ExternalInput":
            if name != partition_name:
                in_names.append(name)
        elif alloc.kind == "ExternalOutput":
            out_names.append(name)
            out_avals.append(
                jax.core.ShapedArray(
                    tuple(alloc.tensor_shape), mybir.dt.np(alloc.dtype)
                )
            )
    n_params = len(in_names)
    all_in_names = tuple(
        in_names + out_names + ([partition_name] if partition_name else [])
    )

    def _body(*args):
        operands = list(args)
        if partition_name is not None:
            operands.append(partition_id_tensor())
        outs = _bass_exec_p.bind(
            *operands,
            out_avals=tuple(out_avals),
            in_names=all_in_names,
            out_names=tuple(out_names),
            lowering_input_output_aliases=(),
            sim_require_finite=True,
            sim_require_nnan=True,
            nc=nc,
        )
        return tuple(outs)

    devices = jax.devices()[:NCORES]
    mesh = Mesh(np.asarray(devices), ("core",))
    shard = NamedSharding(mesh, PartitionSpec("core"))
    n_args = n_params + len(out_names)
    fn = jax.jit(
        shard_map(
            _body,
            mesh=mesh,
            in_specs=(PartitionSpec("core"),) * n_args,
            out_specs=(PartitionSpec("core"),) * len(out_names),
            check_rep=False,
        ),
        keep_unused=True,
    )

    # Cached device-resident operand for the output slot: its content is
    # never observed (the kernel writes every element of y), so one
    # buffer is uploaded once and reused by every call / exec chunk.
    ybuf = jax.device_put(np.zeros((NCORES * RC, D), np.float16), shard)
    jax.block_until_ready(ybuf)

    rt = {
        "fn": fn,
        "shard": shard,
        "devices": devices,
        "ybuf": ybuf,
        "in_names": in_names,
        "jax": jax,
    }
    _CACHE["rt"] = rt
    return rt


def _broadcast_weights(rt, w3):
    """Upload w3 to core 0 once; replicate device-to-device (terminal-side)."""
    jax = rt["jax"]
    devices = rt["devices"]
    w0 = jax.device_put(w3, devices[0])
    wis = [w0] + [jax.device_put(w0, d) for d in devices[1:]]
    return jax.make_array_from_single_device_arrays(
        (NCORES * 3 * 128, G * 128), rt["shard"], wis
    )


def _run(x, weight, alpha, beta, trace=False, **spmd_kwargs):
    rt = _get_runtime()

    w3 = _host_operands(
        np.asarray(weight, dtype=np.float32),
        np.asarray(alpha, dtype=np.float32),
        np.asarray(beta, dtype=np.float32),
    )
    w3g = _broadcast_weights(rt, w3)

    x4 = np.asarray(x, dtype=np.float32).reshape(NCORES, NCH, RC, D)
    fn, ybuf = rt["fn"], rt["ybuf"]
    outs = []
    for j in range(NCH):
        xj = x4[:, j].astype(np.float16).reshape(NCORES * RC, D)
        outs.append(fn(xj, w3g, ybuf))
    for o in outs:
        try:
            o[0].copy_to_host_async()
        except Exception:
            pass
    y32 = np.empty((NCORES, NCH, RC, D), np.float32)
    for j in range(NCH):
        y32[:, j] = np.asarray(outs[j][0]).reshape(NCORES, RC, D)
    y = y32.reshape(B * T, D).reshape(B, T, D)
    return y, _Result()


def kernel(x, weight, alpha, beta):
    y, _ = _run(x, weight, alpha, beta)
    return y


# revision 4
# speedup vs baseline: 4.2642x; 1.3317x over previous
"""HadamardLinear Trainium2 kernel.

Math (per token vector x in R^4096, G=32 groups of IO=128):
  y = beta * ( H^T @ ( (H^T @ (alpha * X)) @W_g grouped ) ) with H the
  normalized 32x32 Hadamard, W_g the per-group ternary-quantized weights.

Device pipeline (per core, RC tokens per exec, fp16 compute / f32 accum):
  CAST: int8 x tile -> fp16 (x is shipped int8 with per-token scales;
        y is linear in x, so the scale folds into the output row scale)
  T1  : PE stride-32-column transposes -> interleaved tiles X_j[g*4+i4, m]
  H1  : PE matmul, activations stationary, H(x)I4*alpha folded in the
        moving matrix -> token-major x_mixed (cols h*128+i)
  T2a : PE transposes of contiguous h-blocks -> feature-major Z_h[i, m]
  MM  : PE matmul, activations stationary, exact ternary sign weights
        -> token-major y_parts (cols h*128+o)
  T2b : PE stride-32-column transposes -> Yint_k[h*4+o4, m]
  H2  : PE matmul, activations stationary, H*beta*scale/32 folded
        -> token-major fp16 output rows
  QNT : per-token row absmax -> s = 126.7/absmax -> int8 quantized rows
        (magic-constant fp16 rounding; s shipped f32 so the host divides
        by exactly the factor the device multiplied with)

Sharding: data-parallel over the 8192 tokens across 8 cores; the three
128x4096 fp16 operand matrices (stacked into one 384x4096 tensor) are
replicated.

Host runner notes: a call's wall time is dominated by the axon tunnel
(~50 MB/s H2D, ~43 MB/s D2H, partially full-duplex), so the runner
 - ships x and y as int8 with per-token f32 scales (4x fewer wire bytes
   than f32; adds ~1% relative error against the 2e-2 budget),
 - uses a jitted shard_map executable built ONCE (no per-call re-trace
   / re-lower / XLA compile),
 - uploads the 3 MB folded weight stack to core 0 only and broadcasts
   it device-to-device (terminal-side, ~free) instead of 8x over the
   tunnel,
 - reuses cached device-resident dummy buffers for the output operands
   (the kernel writes every element, so their content is never
   observed),
 - splits the 8192 tokens into NCH sequential execs so the download of
   chunk i overlaps the upload of chunk i+1.
"""

import sys

if "/opt/trn_rl_repo" not in sys.path:
    sys.path.insert(0, "/opt/trn_rl_repo")

import numpy as np

B, T, D = 4, 2048, 4096
G, IO = 32, 128
NCORES = 8
MC = (B * T) // NCORES      # tokens per core = 1024
RC = 256                    # tokens per core per exec-chunk
NCH = MC // RC              # 4 transfer/exec chunks per call
CHUNK = 128                 # tokens per pipeline chunk
BLK = 128                   # token block (partition dim)
NBLK = 1

_QMAX = 126.7               # quant ceiling with margin so the device-side
                            # reciprocal error can never round a row max
                            # past 127 (int8 overflow would wrap)
_MAGIC = 1536.0             # fp16 round-to-integer constant for |v|<=512


def _hadamard_pm1(n):
    H = np.array([[1.0]], dtype=np.float32)
    while H.shape[0] < n:
        H = np.block([[H, H], [H, -H]])
    return H.astype(np.float32)


_CACHE = {}


def _build_nc():
    if "nc" in _CACHE:
        return _CACHE["nc"]

    import concourse.bass as bass  # noqa: F401
    from concourse import bacc
    import concourse.tile as tile
    import concourse.mybir as mybir
    from concourse.masks import make_identity

    f32 = mybir.dt.float32
    f16 = mybir.dt.float16
    i8 = mybir.dt.int8

    nc = bacc.Bacc("TRN2")
    x_p = nc.declare_dram_parameter("x", [RC, D], i8, isOutput=False)
    w3_p = nc.declare_dram_parameter("w3", [3 * 128, G * 128], f16, isOutput=False)
    y_p = nc.declare_dram_parameter("y", [RC, D], i8, isOutput=True)
    ys_p = nc.declare_dram_parameter("ys", [RC, 1], f32, isOutput=True)

    with tile.TileContext(nc) as tc:
        with (
            tc.tile_pool(name="consts", bufs=1) as consts,
            tc.tile_pool(name="xq", bufs=2) as xq_pool,
            tc.tile_pool(name="xin", bufs=2) as xin_pool,
            tc.tile_pool(name="xint", bufs=2) as xint_pool,
            tc.tile_pool(name="a4", bufs=2) as a4_pool,
            tc.tile_pool(name="z", bufs=2) as z_pool,
            tc.tile_pool(name="yp4", bufs=2) as yp4_pool,
            tc.tile_pool(name="yint", bufs=2) as yint_pool,
            tc.tile_pool(name="yout", bufs=2) as yout_pool,
            tc.tile_pool(name="t1q", bufs=2) as t1q_pool,
            tc.tile_pool(name="yq", bufs=2) as yq_pool,
            tc.tile_pool(name="stat", bufs=8) as stat_pool,
            tc.tile_pool(name="ps_t1", bufs=3, space="PSUM") as ps_t1,
            tc.tile_pool(name="psf", bufs=4, space="PSUM") as psf,
        ):
            ident = consts.tile([128, 128], f16)
            make_identity(nc, ident[:])

            h1w = consts.tile([128, G * 128], f16)
            nc.sync.dma_start(out=h1w[:], in_=w3_p[0:128, :])
            wt = consts.tile([128, G * 128], f16)
            nc.sync.dma_start(out=wt[:], in_=w3_p[128:256, :])
            h2w = consts.tile([128, G * 128], f16)
            nc.sync.dma_start(out=h2w[:], in_=w3_p[256:384, :])

            def copy_engine(idx):
                return nc.vector.tensor_copy if idx % 2 == 0 else nc.scalar.copy

            for c in range(RC // CHUNK):
                # ---- load + dequant-free int8 -> fp16 cast ----
                xq = xq_pool.tile([128, D], i8)
                rows = c * CHUNK
                nc.sync.dma_start(out=xq[:], in_=x_p[rows : rows + BLK, :])
                xin = xin_pool.tile([128, D], f16)
                copy_engine(c)(xin[:], xq[:])

                # ---- T1: token-major -> interleaved feature-major ----
                # X free layout: (j, m) : j*CHUNK + m
                X = xint_pool.tile([128, G * CHUNK], f16)
                for blk in range(NBLK):
                    # cols {j + 32*t} give partition order p = g*4 + i4
                    # (single-stride AP as required for the stationary operand)
                    xv = xin.rearrange("m (g i4 ib) -> m ib (g i4)", g=G, i4=4)
                    for bank in range(4):
                        pt = ps_t1.tile([128, 8 * BLK], f16, tag="pst")
                        for jj in range(8):
                            j = bank * 8 + jj
                            nc.tensor.transpose(
                                pt[:, jj * BLK : (jj + 1) * BLK],
                                xv[:, j, :],
                                ident[:],
                            )
                        dst = X.rearrange("p (j m) -> p j m", j=G)[
                            :, bank * 8 : (bank + 1) * 8, blk * BLK : (blk + 1) * BLK
                        ]
                        src = pt.rearrange("p (jj m) -> p jj m", jj=8)
                        copy_engine(bank + blk)(dst, src)

                # ---- H1 in lhsT form: token-major x_mixed, cols h*128+i ----
                xms = []
                for blk in range(NBLK):
                    xm = z_pool.tile([128, D], f16)
                    xms.append(xm)
                    for jq in range(8):
                        ph = psf.tile([128, 512], f32, tag="psf")
                        for jj in range(4):
                            j = jq * 4 + jj
                            nc.tensor.matmul(
                                ph[:, jj * 128 : (jj + 1) * 128],
                                X[:, j * CHUNK + blk * BLK : j * CHUNK + (blk + 1) * BLK],
                                h1w[:, j * 128 : (j + 1) * 128],
                                start=True,
                                stop=True,
                            )
                        src = ph.rearrange("m (jj h i4) -> m jj h i4", jj=4, h=G)
                        dst = xm.rearrange("m (h i4 j) -> m j h i4", h=G, i4=4)[
                            :, jq * 4 : (jq + 1) * 4, :, :
                        ]
                        copy_engine(jq + blk)(dst, src)

                # ---- T2a: x_mixed -> feature-major Z_h[i, m] ----
                Z = yint_pool.tile([128, G * CHUNK], f16)
                for blk in range(NBLK):
                    for hq in range(4):
                        pz = ps_t1.tile([128, 8 * BLK], f16, tag="pst")
                        for hh in range(8):
                            h = hq * 8 + hh
                            nc.tensor.transpose(
                                pz[:, hh * BLK : (hh + 1) * BLK],
                                xms[blk][:, h * 128 : (h + 1) * 128],
                                ident[:],
                            )
                        dst = Z.rearrange("i (h m) -> i h m", h=G)[
                            :, hq * 8 : (hq + 1) * 8, blk * BLK : (blk + 1) * BLK
                        ]
                        src = pz.rearrange("i (hh m) -> i hh m", hh=8)
                        copy_engine(hq + blk + 1)(dst, src)

                # ---- MM in lhsT form: token-major y_parts, cols h*128+o ----
                yps = []
                for blk in range(NBLK):
                    yp = a4_pool.tile([128, D], f16)
                    yps.append(yp)
                    for hq in range(8):
                        pm = psf.tile([128, 512], f32, tag="psf")
                        for hh in range(4):
                            h = hq * 4 + hh
                            nc.tensor.matmul(
                                pm[:, hh * 128 : (hh + 1) * 128],
                                Z[:, h * CHUNK + blk * BLK : h * CHUNK + (blk + 1) * BLK],
                                wt[:, h * 128 : (h + 1) * 128],
                                start=True,
                                stop=True,
                            )
                        # cols h*128+o are contiguous per h-block
                        copy_engine(hq + blk)(
                            yp[:, hq * 512 : (hq + 1) * 512], pm[:]
                        )

                # ---- T2b: y_parts -> Yint_k[p2=h*4+o4, m] (stride-32 cols) ----
                Yint = yp4_pool.tile([128, G * CHUNK], f16)
                for blk in range(NBLK):
                    ypv = yps[blk].rearrange("m (h o4 kb) -> m kb (h o4)", h=G, o4=4)
                    for kq in range(4):
                        pyi = ps_t1.tile([128, 8 * BLK], f16, tag="pst")
                        for kk in range(8):
                            k = kq * 8 + kk
                            nc.tensor.transpose(
                                pyi[:, kk * BLK : (kk + 1) * BLK],
                                ypv[:, k, :],
                                ident[:],
                            )
                        dst = Yint.rearrange("p (k m) -> p k m", k=G)[
                            :, kq * 8 : (kq + 1) * 8, blk * BLK : (blk + 1) * BLK
                        ]
                        src = pyi.rearrange("p (kk m) -> p kk m", kk=8)
                        copy_engine(kq + blk)(dst, src)

                # ---- H2 (token-major fp16 rows) ----
                for blk in range(NBLK):
                    yout = yout_pool.tile([128, D], f16)
                    for kg in range(8):
                        p2 = psf.tile([128, 4 * 128], f32, tag="psf")
                        for kk in range(4):
                            k = kg * 4 + kk
                            nc.tensor.matmul(
                                p2[:, kk * 128 : (kk + 1) * 128],
                                Yint[:, k * CHUNK + blk * BLK : k * CHUNK + (blk + 1) * BLK],
                                h2w[:, k * 128 : (k + 1) * 128],
                                start=True,
                                stop=True,
                            )
                        src = p2.rearrange("p (kk hp o4p) -> p kk hp o4p", kk=4, hp=G)
                        dst = yout.rearrange(
                            "m (hp o4p k) -> m k hp o4p", hp=G, o4p=4
                        )[:, kg * 4 : (kg + 1) * 4, :, :]
                        copy_engine(kg + blk)(dst, src)

                    # ---- QNT: per-token absmax -> s -> int8 rows ----
                    m = stat_pool.tile([128, 1], f32)
                    nc.vector.tensor_reduce(
                        out=m[:],
                        in_=yout[:],
                        axis=mybir.AxisListType.X,
                        op=mybir.AluOpType.max,
                        apply_absolute_value=True,
                    )
                    m2 = stat_pool.tile([128, 1], f32)
                    nc.vector.tensor_scalar_max(out=m2[:], in0=m[:], scalar1=1e-6)
                    rinv = stat_pool.tile([128, 1], f32)
                    nc.vector.reciprocal(out=rinv[:], in_=m2[:])
                    s = stat_pool.tile([128, 1], f32)
                    nc.vector.tensor_scalar_mul(
                        out=s[:], in0=rinv[:], scalar1=_QMAX
                    )
                    rows = c * CHUNK + blk * BLK
                    nc.sync.dma_start(out=ys_p[rows : rows + BLK, :], in_=s[:])

                    # t1 = round(yout * s) + MAGIC, exact integers in fp16
                    t1 = t1q_pool.tile([128, D], f16)
                    nc.scalar.activation(
                        out=t1[:],
                        in_=yout[:],
                        func=mybir.ActivationFunctionType.Copy,
                        scale=s[:],
                        bias=_MAGIC,
                    )
                    yq = yq_pool.tile([128, D], i8)
                    nc.vector.tensor_scalar(
                        out=yq[:],
                        in0=t1[:],
                        scalar1=_MAGIC,
                        scalar2=None,
                        op0=mybir.AluOpType.subtract,
                    )
                    nc.sync.dma_start(out=y_p[rows : rows + BLK, :], in_=yq[:])

    nc.finalize()
    _CACHE["nc"] = nc
    return nc


def _host_operands(weight, alpha, beta):
    """Fold quantization, Hadamards, alpha/beta into 3 device matrices."""
    Hr = _hadamard_pm1(G)  # +-1, exact in fp16
    scale = max(float(np.abs(weight).mean()), 1e-8)
    wq_sign = np.clip(np.round(weight / scale), -1.0, 1.0).astype(np.float32)
    S = scale / 32.0  # the two 1/sqrt(32) factors + ternary scale

    # h1w[p=(g*4+i4), j, q=(h*4+i4p)] = delta(i4,i4p) * Hr[g,h] * alpha[g, i4*32+j]
    h1w = np.zeros((G, 4, G, G, 4), dtype=np.float32)  # g, i4, j, h, i4p
    for i4 in range(4):
        a = alpha[:, i4 * 32 : (i4 + 1) * 32].astype(np.float32)  # [g, j]
        h1w[:, i4, :, :, i4] = a[:, :, None] * Hr[:, None, :]
    h1w = h1w.reshape(128, G * 128)

    # wt[i, h, o] = wq_sign[h, o, i]
    wt = np.ascontiguousarray(np.transpose(wq_sign, (2, 0, 1))).reshape(128, G * 128)

    # h2w[p'=(o4*32+h), k, q'=(hp*4+o4p)] = delta(o4,o4p)*Hr[h,hp]*beta[hp,o4*32+k]*S
    h2w = np.zeros((4, G, G, G, 4), dtype=np.float32)  # o4, h, k, hp, o4p
    for o4 in range(4):
        b = beta[:, o4 * 32 : (o4 + 1) * 32].astype(np.float32) * S  # [hp, k]
        # Hr[h, hp] * b[hp, k] -> [h, k, hp]
        h2w[o4, :, :, :, o4] = Hr[:, None, :] * b.T[None, :, :]
    # device rows use p2 = h*4 + o4 (T2b transpose partition order)
    h2w = np.transpose(h2w, (1, 0, 2, 3, 4)).reshape(128, G * 128)

    w3 = np.concatenate([h1w, wt, h2w], axis=0).astype(np.float16)
    return w3


class _Result:
    """Minimal result shim for test harnesses (no NTFF trace available)."""

    exec_time_ns = None
    mean_exec_time_ns = None


def _get_runtime():
    """Build (once) the jitted shard_map executable and cached device buffers."""
    if "rt" in _CACHE:
        return _CACHE["rt"]

    nc = _build_nc()

    import jax
    from jax.sharding import Mesh, PartitionSpec, NamedSharding
    from jax.experimental.shard_map import shard_map
    from concourse.bass2jax import (
        _bass_exec_p,
        install_neuronx_cc_hook,
        partition_id_tensor,
    )
    import concourse.mybir as mybir

    install_neuronx_cc_hook()

    partition_name = (
        nc.partition_id_tensor.name if nc.partition_id_tensor else None
    )
    in_names, out_names, out_avals = [], [], []
    for alloc in nc.m.functions[0].allocations:
        if not isinstance(alloc, mybir.MemoryLocationSet):
            continue
        name = alloc.memorylocations[0].name
        if alloc.kind == "ExternalInput":
            if name != partition_name:
                in_names.append(name)
        elif alloc.kind == "ExternalOutput":
            out_names.append(name)
            out_avals.append(
                jax.core.ShapedArray(
                    tuple(alloc.tensor_shape), mybir.dt.np(alloc.dtype)
                )
            )
    n_params = len(in_names)
    all_in_names = tuple(
        in_names + out_names + ([partition_name] if partition_name else [])
    )

    def _body(*args):
        operands = list(args)
        if partition_name is not None:
            operands.append(partition_id_tensor())
        outs = _bass_exec_p.bind(
            *operands,
            out_avals=tuple(out_avals),
            in_names=all_in_names,
            out_names=tuple(out_names),
            lowering_input_output_aliases=(),
            sim_require_finite=True,
            sim_require_nnan=True,
            nc=nc,
        )
        return tuple(outs)

    devices = jax.devices()[:NCORES]
    mesh = Mesh(np.asarray(devices), ("core",))
    shard = NamedSharding(mesh, PartitionSpec("core"))
    n_args = n_params + len(out_names)
    fn = jax.jit(
        shard_map(
            _body,
            mesh=mesh,
            in_specs=(PartitionSpec("core"),) * n_args,
            out_specs=(PartitionSpec("core"),) * len(out_names),
            check_rep=False,
        ),
        keep_unused=True,
    )

    # Cached device-resident operands for the output slots: their content
    # is never observed (the kernel writes every element), so they are
    # uploaded once and reused by every call / exec chunk.
    obufs = []
    for nm, av in zip(out_names, out_avals):
        obufs.append(
            jax.device_put(
                np.zeros((NCORES * av.shape[0],) + tuple(av.shape[1:]), av.dtype),
                shard,
            )
        )
    jax.block_until_ready(obufs)

    rt = {
        "fn": fn,
        "shard": shard,
        "devices": devices,
        "obufs": obufs,
        "in_names": in_names,
        "out_names": out_names,
        "jax": jax,
    }
    _CACHE["rt"] = rt
    return rt


def _broadcast_weights(rt, w3):
    """Upload w3 to core 0 once; replicate device-to-device (terminal-side)."""
    jax = rt["jax"]
    devices = rt["devices"]
    w0 = jax.device_put(w3, devices[0])
    wis = [w0] + [jax.device_put(w0, d) for d in devices[1:]]
    return jax.make_array_from_single_device_arrays(
        (NCORES * 3 * 128, G * 128), rt["shard"], wis
    )


def _run(x, weight, alpha, beta, trace=False, **spmd_kwargs):
    rt = _get_runtime()

    w3 = _host_operands(
        np.asarray(weight, dtype=np.float32),
        np.asarray(alpha, dtype=np.float32),
        np.asarray(beta, dtype=np.float32),
    )
    w3g = _broadcast_weights(rt, w3)

    x4 = np.asarray(x, dtype=np.float32).reshape(NCORES, NCH, RC, D)
    m_x = np.maximum(np.abs(x4).max(axis=3), 1e-30)       # [NCORES, NCH, RC]
    s_x = (127.0 / m_x).astype(np.float32)

    fn = rt["fn"]
    obufs = rt["obufs"]
    outs = []
    for j in range(NCH):
        q = np.rint(x4[:, j] * s_x[:, j, :, None])
        xq = q.astype(np.int8).reshape(NCORES * RC, D)
        outs.append(fn(xq, w3g, *obufs))
    for o in outs:
        for arr in o:
            try:
                arr.copy_to_host_async()
            except Exception:
                pass
    y32 = np.empty((NCORES, NCH, RC, D), np.float32)
    for j in range(NCH):
        s_dev = np.asarray(outs[j][1]).reshape(NCORES, RC)       # 126.7/absmax
        yq = np.asarray(outs[j][0]).reshape(NCORES, RC, D)       # int8
        row_scale = m_x[:, j] / (127.0 * s_dev)                  # [NCORES, RC]
        np.multiply(yq, row_scale[:, :, None], out=y32[:, j])
    y = y32.reshape(B * T, D).reshape(B, T, D)
    return y, _Result()


def kernel(x, weight, alpha, beta):
    y, _ = _run(x, weight, alpha, beta)
    return y


# revision 5
# speedup vs baseline: 5.9858x; 1.4037x over previous
"""HadamardLinear Trainium2 kernel.

Math (per token vector x in R^4096, G=32 groups of IO=128):
  y = beta * ( H^T @ ( (H^T @ (alpha * X)) @W_g grouped ) ) with H the
  normalized 32x32 Hadamard, W_g the per-group ternary-quantized weights.

Device pipeline (per core, RC tokens per exec, fp16 compute / f32 accum):
  CAST: int8 x tile -> fp16 (x is shipped int8 with per-token scales;
        y is linear in x, so the scale folds into the output row scale)
  T1  : PE stride-32-column transposes -> interleaved tiles X_j[g*4+i4, m]
  H1  : PE matmul, activations stationary, H(x)I4*alpha folded in the
        moving matrix -> token-major x_mixed (cols h*128+i)
  T2a : PE transposes of contiguous h-blocks -> feature-major Z_h[i, m]
  MM  : PE matmul, activations stationary, exact ternary sign weights
        -> token-major y_parts (cols h*128+o)
  T2b : PE stride-32-column transposes -> Yint_k[h*4+o4, m]
  H2  : PE matmul, activations stationary, H*beta*scale/32 folded
        -> token-major fp16 output rows
  QNT : per-token row absmax -> s = 126.7/absmax -> int8 quantized rows
        (magic-constant fp16 rounding; s shipped f32 so the host divides
        by exactly the factor the device multiplied with)

Sharding: data-parallel over the 8192 tokens across 8 cores; the three
128x4096 fp16 operand matrices (stacked into one 384x4096 tensor) are
replicated.

Host runner notes: a call's wall time is dominated by the axon tunnel
(~50 MB/s H2D, ~43 MB/s D2H, partially full-duplex), so the runner
 - ships x and y as int8 with per-token f32 scales (4x fewer wire bytes
   than f32; adds ~1% relative error against the 2e-2 budget),
 - uses a jitted shard_map executable built ONCE (no per-call re-trace
   / re-lower / XLA compile),
 - uploads the 3 MB folded weight stack to core 0 only and broadcasts
   it device-to-device (terminal-side, ~free) instead of 8x over the
   tunnel,
 - reuses cached device-resident dummy buffers for the output operands
   (the kernel writes every element, so their content is never
   observed),
 - splits the 8192 tokens into NCH sequential execs so the download of
   chunk i overlaps the upload of chunk i+1.
"""

import sys

if "/opt/trn_rl_repo" not in sys.path:
    sys.path.insert(0, "/opt/trn_rl_repo")

import numpy as np

B, T, D = 4, 2048, 4096
G, IO = 32, 128
NCORES = 8
MC = (B * T) // NCORES      # tokens per core = 1024
RC = 256                    # tokens per core per exec-chunk
NCH = MC // RC              # 4 transfer/exec chunks per call
CHUNK = 128                 # tokens per pipeline chunk
BLK = 128                   # token block (partition dim)
NBLK = 1

_QMAX = 126.7               # quant ceiling with margin so the device-side
                            # reciprocal error can never round a row max
                            # past 127 (int8 overflow would wrap)
_MAGIC = 1536.0             # fp16 round-to-integer constant for |v|<=512


def _hadamard_pm1(n):
    H = np.array([[1.0]], dtype=np.float32)
    while H.shape[0] < n:
        H = np.block([[H, H], [H, -H]])
    return H.astype(np.float32)


_CACHE = {}


def _build_nc():
    if "nc" in _CACHE:
        return _CACHE["nc"]

    import concourse.bass as bass  # noqa: F401
    from concourse import bacc
    import concourse.tile as tile
    import concourse.mybir as mybir
    from concourse.masks import make_identity

    f32 = mybir.dt.float32
    f16 = mybir.dt.float16
    i8 = mybir.dt.int8

    nc = bacc.Bacc("TRN2")
    x_p = nc.declare_dram_parameter("x", [RC, D], i8, isOutput=False)
    w3_p = nc.declare_dram_parameter("w3", [3 * 128, G * 128], f16, isOutput=False)
    y_p = nc.declare_dram_parameter("y", [RC, D], i8, isOutput=True)
    ys_p = nc.declare_dram_parameter("ys", [RC, 1], f32, isOutput=True)

    with tile.TileContext(nc) as tc:
        with (
            tc.tile_pool(name="consts", bufs=1) as consts,
            tc.tile_pool(name="xq", bufs=2) as xq_pool,
            tc.tile_pool(name="xin", bufs=2) as xin_pool,
            tc.tile_pool(name="xint", bufs=2) as xint_pool,
            tc.tile_pool(name="a4", bufs=2) as a4_pool,
            tc.tile_pool(name="z", bufs=2) as z_pool,
            tc.tile_pool(name="yp4", bufs=2) as yp4_pool,
            tc.tile_pool(name="yint", bufs=2) as yint_pool,
            tc.tile_pool(name="yout", bufs=2) as yout_pool,
            tc.tile_pool(name="t1q", bufs=2) as t1q_pool,
            tc.tile_pool(name="yq", bufs=2) as yq_pool,
            tc.tile_pool(name="stat", bufs=8) as stat_pool,
            tc.tile_pool(name="ps_t1", bufs=3, space="PSUM") as ps_t1,
            tc.tile_pool(name="psf", bufs=4, space="PSUM") as psf,
        ):
            ident = consts.tile([128, 128], f16)
            make_identity(nc, ident[:])

            h1w = consts.tile([128, G * 128], f16)
            nc.sync.dma_start(out=h1w[:], in_=w3_p[0:128, :])
            wt = consts.tile([128, G * 128], f16)
            nc.sync.dma_start(out=wt[:], in_=w3_p[128:256, :])
            h2w = consts.tile([128, G * 128], f16)
            nc.sync.dma_start(out=h2w[:], in_=w3_p[256:384, :])

            def copy_engine(idx):
                return nc.vector.tensor_copy if idx % 2 == 0 else nc.scalar.copy

            for c in range(RC // CHUNK):
                # ---- load + dequant-free int8 -> fp16 cast ----
                xq = xq_pool.tile([128, D], i8)
                rows = c * CHUNK
                nc.sync.dma_start(out=xq[:], in_=x_p[rows : rows + BLK, :])
                xin = xin_pool.tile([128, D], f16)
                copy_engine(c)(xin[:], xq[:])

                # ---- T1: token-major -> interleaved feature-major ----
                # X free layout: (j, m) : j*CHUNK + m
                X = xint_pool.tile([128, G * CHUNK], f16)
                for blk in range(NBLK):
                    # cols {j + 32*t} give partition order p = g*4 + i4
                    # (single-stride AP as required for the stationary operand)
                    xv = xin.rearrange("m (g i4 ib) -> m ib (g i4)", g=G, i4=4)
                    for bank in range(4):
                        pt = ps_t1.tile([128, 8 * BLK], f16, tag="pst")
                        for jj in range(8):
                            j = bank * 8 + jj
                            nc.tensor.transpose(
                                pt[:, jj * BLK : (jj + 1) * BLK],
                                xv[:, j, :],
                                ident[:],
                            )
                        dst = X.rearrange("p (j m) -> p j m", j=G)[
                            :, bank * 8 : (bank + 1) * 8, blk * BLK : (blk + 1) * BLK
                        ]
                        src = pt.rearrange("p (jj m) -> p jj m", jj=8)
                        copy_engine(bank + blk)(dst, src)

                # ---- H1 in lhsT form: token-major x_mixed, cols h*128+i ----
                xms = []
                for blk in range(NBLK):
                    xm = z_pool.tile([128, D], f16)
                    xms.append(xm)
                    for jq in range(8):
                        ph = psf.tile([128, 512], f32, tag="psf")
                        for jj in range(4):
                            j = jq * 4 + jj
                            nc.tensor.matmul(
                                ph[:, jj * 128 : (jj + 1) * 128],
                                X[:, j * CHUNK + blk * BLK : j * CHUNK + (blk + 1) * BLK],
                                h1w[:, j * 128 : (j + 1) * 128],
                                start=True,
                                stop=True,
                            )
                        src = ph.rearrange("m (jj h i4) -> m jj h i4", jj=4, h=G)
                        dst = xm.rearrange("m (h i4 j) -> m j h i4", h=G, i4=4)[
                            :, jq * 4 : (jq + 1) * 4, :, :
                        ]
                        copy_engine(jq + blk)(dst, src)

                # ---- T2a: x_mixed -> feature-major Z_h[i, m] ----
                Z = yint_pool.tile([128, G * CHUNK], f16)
                for blk in range(NBLK):
                    for hq in range(4):
                        pz = ps_t1.tile([128, 8 * BLK], f16, tag="pst")
                        for hh in range(8):
                            h = hq * 8 + hh
                            nc.tensor.transpose(
                                pz[:, hh * BLK : (hh + 1) * BLK],
                                xms[blk][:, h * 128 : (h + 1) * 128],
                                ident[:],
                            )
                        dst = Z.rearrange("i (h m) -> i h m", h=G)[
                            :, hq * 8 : (hq + 1) * 8, blk * BLK : (blk + 1) * BLK
                        ]
                        src = pz.rearrange("i (hh m) -> i hh m", hh=8)
                        copy_engine(hq + blk + 1)(dst, src)

                # ---- MM in lhsT form: token-major y_parts, cols h*128+o ----
                yps = []
                for blk in range(NBLK):
                    yp = a4_pool.tile([128, D], f16)
                    yps.append(yp)
                    for hq in range(8):
                        pm = psf.tile([128, 512], f32, tag="psf")
                        for hh in range(4):
                            h = hq * 4 + hh
                            nc.tensor.matmul(
                                pm[:, hh * 128 : (hh + 1) * 128],
                                Z[:, h * CHUNK + blk * BLK : h * CHUNK + (blk + 1) * BLK],
                                wt[:, h * 128 : (h + 1) * 128],
                                start=True,
                                stop=True,
                            )
                        # cols h*128+o are contiguous per h-block
                        copy_engine(hq + blk)(
                            yp[:, hq * 512 : (hq + 1) * 512], pm[:]
                        )

                # ---- T2b: y_parts -> Yint_k[p2=h*4+o4, m] (stride-32 cols) ----
                Yint = yp4_pool.tile([128, G * CHUNK], f16)
                for blk in range(NBLK):
                    ypv = yps[blk].rearrange("m (h o4 kb) -> m kb (h o4)", h=G, o4=4)
                    for kq in range(4):
                        pyi = ps_t1.tile([128, 8 * BLK], f16, tag="pst")
                        for kk in range(8):
                            k = kq * 8 + kk
                            nc.tensor.transpose(
                                pyi[:, kk * BLK : (kk + 1) * BLK],
                                ypv[:, k, :],
                                ident[:],
                            )
                        dst = Yint.rearrange("p (k m) -> p k m", k=G)[
                            :, kq * 8 : (kq + 1) * 8, blk * BLK : (blk + 1) * BLK
                        ]
                        src = pyi.rearrange("p (kk m) -> p kk m", kk=8)
                        copy_engine(kq + blk)(dst, src)

                # ---- H2 (token-major fp16 rows) ----
                for blk in range(NBLK):
                    yout = yout_pool.tile([128, D], f16)
                    for kg in range(8):
                        p2 = psf.tile([128, 4 * 128], f32, tag="psf")
                        for kk in range(4):
                            k = kg * 4 + kk
                            nc.tensor.matmul(
                                p2[:, kk * 128 : (kk + 1) * 128],
                                Yint[:, k * CHUNK + blk * BLK : k * CHUNK + (blk + 1) * BLK],
                                h2w[:, k * 128 : (k + 1) * 128],
                                start=True,
                                stop=True,
                            )
                        src = p2.rearrange("p (kk hp o4p) -> p kk hp o4p", kk=4, hp=G)
                        dst = yout.rearrange(
                            "m (hp o4p k) -> m k hp o4p", hp=G, o4p=4
                        )[:, kg * 4 : (kg + 1) * 4, :, :]
                        copy_engine(kg + blk)(dst, src)

                    # ---- QNT: per-token absmax -> s -> int8 rows ----
                    m = stat_pool.tile([128, 1], f32)
                    nc.vector.tensor_reduce(
                        out=m[:],
                        in_=yout[:],
                        axis=mybir.AxisListType.X,
                        op=mybir.AluOpType.max,
                        apply_absolute_value=True,
                    )
                    m2 = stat_pool.tile([128, 1], f32)
                    nc.vector.tensor_scalar_max(out=m2[:], in0=m[:], scalar1=1e-6)
                    rinv = stat_pool.tile([128, 1], f32)
                    nc.vector.reciprocal(out=rinv[:], in_=m2[:])
                    s = stat_pool.tile([128, 1], f32)
                    nc.vector.tensor_scalar_mul(
                        out=s[:], in0=rinv[:], scalar1=_QMAX
                    )
                    rows = c * CHUNK + blk * BLK
                    nc.sync.dma_start(out=ys_p[rows : rows + BLK, :], in_=s[:])

                    # t1 = round(yout * s) + MAGIC, exact integers in fp16
                    t1 = t1q_pool.tile([128, D], f16)
                    nc.scalar.activation(
                        out=t1[:],
                        in_=yout[:],
                        func=mybir.ActivationFunctionType.Copy,
                        scale=s[:],
                        bias=_MAGIC,
                    )
                    yq = yq_pool.tile([128, D], i8)
                    nc.vector.tensor_scalar(
                        out=yq[:],
                        in0=t1[:],
                        scalar1=_MAGIC,
                        scalar2=None,
                        op0=mybir.AluOpType.subtract,
                    )
                    nc.sync.dma_start(out=y_p[rows : rows + BLK, :], in_=yq[:])

    nc.finalize()
    _CACHE["nc"] = nc
    return nc


def _host_operands(weight, alpha, beta):
    """Fold quantization, Hadamards, alpha/beta into 3 device matrices."""
    Hr = _hadamard_pm1(G)  # +-1, exact in fp16
    scale = max(float(np.abs(weight).mean()), 1e-8)
    wq_sign = np.clip(np.round(weight / scale), -1.0, 1.0).astype(np.float32)
    S = scale / 32.0  # the two 1/sqrt(32) factors + ternary scale

    # h1w[p=(g*4+i4), j, q=(h*4+i4p)] = delta(i4,i4p) * Hr[g,h] * alpha[g, i4*32+j]
    h1w = np.zeros((G, 4, G, G, 4), dtype=np.float32)  # g, i4, j, h, i4p
    for i4 in range(4):
        a = alpha[:, i4 * 32 : (i4 + 1) * 32].astype(np.float32)  # [g, j]
        h1w[:, i4, :, :, i4] = a[:, :, None] * Hr[:, None, :]
    h1w = h1w.reshape(128, G * 128)

    # wt[i, h, o] = wq_sign[h, o, i]
    wt = np.ascontiguousarray(np.transpose(wq_sign, (2, 0, 1))).reshape(128, G * 128)

    # h2w[p'=(o4*32+h), k, q'=(hp*4+o4p)] = delta(o4,o4p)*Hr[h,hp]*beta[hp,o4*32+k]*S
    h2w = np.zeros((4, G, G, G, 4), dtype=np.float32)  # o4, h, k, hp, o4p
    for o4 in range(4):
        b = beta[:, o4 * 32 : (o4 + 1) * 32].astype(np.float32) * S  # [hp, k]
        # Hr[h, hp] * b[hp, k] -> [h, k, hp]
        h2w[o4, :, :, :, o4] = Hr[:, None, :] * b.T[None, :, :]
    # device rows use p2 = h*4 + o4 (T2b transpose partition order)
    h2w = np.transpose(h2w, (1, 0, 2, 3, 4)).reshape(128, G * 128)

    w3 = np.concatenate([h1w, wt, h2w], axis=0).astype(np.float16)
    return w3


class _Result:
    """Minimal result shim for test harnesses (no NTFF trace available)."""

    exec_time_ns = None
    mean_exec_time_ns = None


def _get_runtime():
    """Build (once) the jitted shard_map executable and cached device buffers."""
    if "rt" in _CACHE:
        return _CACHE["rt"]

    nc = _build_nc()

    import jax
    from jax.sharding import Mesh, PartitionSpec, NamedSharding
    from jax.experimental.shard_map import shard_map
    from concourse.bass2jax import (
        _bass_exec_p,
        install_neuronx_cc_hook,
        partition_id_tensor,
    )
    import concourse.mybir as mybir

    install_neuronx_cc_hook()

    partition_name = (
        nc.partition_id_tensor.name if nc.partition_id_tensor else None
    )
    in_names, out_names, out_avals = [], [], []
    for alloc in nc.m.functions[0].allocations:
        if not isinstance(alloc, mybir.MemoryLocationSet):
            continue
        name = alloc.memorylocations[0].name
        if alloc.kind == "ExternalInput":
            if name != partition_name:
                in_names.append(name)
        elif alloc.kind == "ExternalOutput":
            out_names.append(name)
            out_avals.append(
                jax.core.ShapedArray(
                    tuple(alloc.tensor_shape), mybir.dt.np(alloc.dtype)
                )
            )
    n_params = len(in_names)
    all_in_names = tuple(
        in_names + out_names + ([partition_name] if partition_name else [])
    )

    def _body(*args):
        operands = list(args)
        if partition_name is not None:
            operands.append(partition_id_tensor())
        outs = _bass_exec_p.bind(
            *operands,
            out_avals=tuple(out_avals),
            in_names=all_in_names,
            out_names=tuple(out_names),
            lowering_input_output_aliases=(),
            sim_require_finite=True,
            sim_require_nnan=True,
            nc=nc,
        )
        return tuple(outs)

    devices = jax.devices()[:NCORES]
    mesh = Mesh(np.asarray(devices), ("core",))
    shard = NamedSharding(mesh, PartitionSpec("core"))
    n_args = n_params + len(out_names)
    fn = jax.jit(
        shard_map(
            _body,
            mesh=mesh,
            in_specs=(PartitionSpec("core"),) * n_args,
            out_specs=(PartitionSpec("core"),) * len(out_names),
            check_rep=False,
        ),
        keep_unused=True,
    )

    # Cached device-resident operands for the output slots: their content
    # is never observed (the kernel writes every element), so they are
    # uploaded once and reused by every call / exec chunk.
    obufs = []
    for nm, av in zip(out_names, out_avals):
        obufs.append(
            jax.device_put(
                np.zeros((NCORES * av.shape[0],) + tuple(av.shape[1:]), av.dtype),
                shard,
            )
        )
    jax.block_until_ready(obufs)

    rt = {
        "fn": fn,
        "shard": shard,
        "devices": devices,
        "obufs": obufs,
        "in_names": in_names,
        "out_names": out_names,
        "jax": jax,
    }
    _CACHE["rt"] = rt
    return rt


def _broadcast_weights(rt, w3):
    """Upload w3 to core 0 once; replicate device-to-device (terminal-side)."""
    jax = rt["jax"]
    devices = rt["devices"]
    w0 = jax.device_put(w3, devices[0])
    wis = [w0] + [jax.device_put(w0, d) for d in devices[1:]]
    return jax.make_array_from_single_device_arrays(
        (NCORES * 3 * 128, G * 128), rt["shard"], wis
    )


def _run(x, weight, alpha, beta, trace=False, **spmd_kwargs):
    rt = _get_runtime()

    w3 = _host_operands(
        np.asarray(weight, dtype=np.float32),
        np.asarray(alpha, dtype=np.float32),
        np.asarray(beta, dtype=np.float32),
    )
    w3g = _broadcast_weights(rt, w3)

    x4 = np.asarray(x, dtype=np.float32).reshape(NCORES, NCH, RC, D)
    fn = rt["fn"]
    obufs = rt["obufs"]
    outs = []
    m_list = []
    scratch = np.empty((NCORES, RC, D), np.float32)
    for j in range(NCH):
        xj = x4[:, j]
        # per-token absmax via max/-min: two read passes, no 33 MB temp
        mj = np.maximum(xj.max(axis=2), -xj.min(axis=2))  # [NCORES, RC]
        np.maximum(mj, 1e-30, out=mj)
        m_list.append(mj)
        sj = np.float32(127.0) / mj
        np.multiply(xj, sj[:, :, None], out=scratch)
        np.rint(scratch, out=scratch)
        xq = scratch.astype(np.int8).reshape(NCORES * RC, D)
        o = fn(xq, w3g, *obufs)
        outs.append(o)
        for arr in o:
            try:
                arr.copy_to_host_async()
            except Exception:
                pass
    y32 = np.empty((NCORES, NCH, RC, D), np.float32)
    for j in range(NCH):
        s_dev = np.asarray(outs[j][1]).reshape(NCORES, RC)       # 126.7/absmax
        yq = np.asarray(outs[j][0]).reshape(NCORES, RC, D)       # int8
        row_scale = m_list[j] / (127.0 * s_dev)                  # [NCORES, RC]
        np.multiply(yq, row_scale[:, :, None], out=y32[:, j])
    y = y32.reshape(B * T, D).reshape(B, T, D)
    return y, _Result()


def kernel(x, weight, alpha, beta):
    y, _ = _run(x, weight, alpha, beta)
    return y
